# revision 30
# baseline (speedup 1.0000x reference)
"""LIF (leaky integrate-and-fire) forward kernel for Trainium2, 8 NeuronCores.

Recurrence (per element of [B, N], serial over T):
    v_t = DECAY * v_{t-1} * (1 - s_{t-1}) + x_t
    s_t = (v_t > THRESHOLD)

State carried as v (pre-reset membrane, fp16). One fused custom-DVE op per
step (registered at build time via the public dve_ops extension point):
    v_t = (v_{t-1} <= THR) * v_{t-1} * DECAY + x_t
with a hand-packed 2X_1PORT uop variant (element A on ALU slices 0-3,
element B from the SRC_*_HI packed fp16 halves on slices 4-7) and
perf_max=1 set on each emitted instruction so the engine may engage it.
Spikes: DVE (v > THR) -> {0,1} fp8 for the first SW columns, ScalarE
Sign(v-THR) -> {-1,0,1} fp8 for the rest; host decodes (y > 0) for both.
Inputs stream as step-pairs on the SP HWDGE ring; output DMA is issued from
the Activation engine's HWDGE ring so it never head-of-line blocks the
input stream (final step drains via the then-idle SP ring).

Input is converted to fp16 on the host (halves input HBM traffic; measured
spike-flip impact 2211/67M elements, rel err 0.0097 < 2e-2 gate).

Sharding: batch dim (128) split 16 rows/core across 8 cores; per-core,
per-step slab is a contiguous block viewed as [128 partitions, 2048].
"""

import copy

import numpy as np

import concourse.bacc as bacc
import concourse.mybir as mybir
from concourse.tile import TileContext
from concourse.bass_utils import run_bass_kernel_spmd
import concourse.dve_ops as dve_ops
from concourse.dve_ops import DveOp
from concourse.dve_spec import Spec, Src0, Src1, C0, C1, lower
from concourse.dve_uop import (
    AluInp, AluOp as UAluOp, DelayInp, DveOpSpec, InpSel, OutPath, OutSel,
    UopDpConfig,
)

T, B, N = 32, 128, 16384
N_CORES = 8
B_SH = B // N_CORES          # 16 batch rows per core
S = B_SH * N                 # 262144 elements per core per time step
P = 128                      # SBUF partitions
F = S // P                   # 2048 free-dim elements
SW = 768                     # spike columns computed on DVE (rest on Act)
DECAY = 0.2
THR = 0.3

TRACE = False                # set True (e.g. from test.py) to capture a profile

_BUILT = {}


def _lif_ref(in0, in1, s0, s1, imm2):
    v = np.where(np.asarray(in0, dtype=np.float32) <= np.float32(s1),
                 np.asarray(in0, dtype=np.float32), np.float32(0.0))
    return v * np.float32(s0) + np.asarray(in1, dtype=np.float32)


def _build_2x_uop(reg_uop):
    """Hand-packed 2X_1PORT variant: element A on ALU slices 0-3, element B
    (from the SRC_*_HI packed half) on slices 4-7; A's result rides delay
    lane 3 from stage 4 and is emitted on WR0_LO, B's on WR0_HI."""
    u2 = copy.deepcopy(reg_uop)
    u2.inp = [InpSel.ZERO] * 8
    u2.inp_enable = [0] * 8
    for sel, slot in ((InpSel.CONST_1, 1), (InpSel.SRC_0, 2), (InpSel.CONST_0, 3),
                      (InpSel.SRC_1, 4), (InpSel.SRC_0_HI, 5), (InpSel.SRC_1_HI, 6)):
        u2.enable_input(sel, slot)
    D = AluInp
    dp = [UopDpConfig() for _ in range(8)]
    dp[0].enable_alu(UAluOp.IS_GE, D.PREV_DELAY_0, D.PREV_DELAY_1) \
        .pass_through_delay(0, 1, 2, 3, 4, 5)
    dp[1].enable_alu(UAluOp.MULTIPLY, D.PREV_ALU_OUT, D.PREV_DELAY_1) \
        .pass_through_delay(0, 2, 3, 4, 5)
    dp[2].enable_alu(UAluOp.MULTIPLY, D.PREV_ALU_OUT, D.PREV_DELAY_2) \
        .pass_through_delay(0, 2, 3, 4, 5)
    dp[3].enable_alu(UAluOp.ADD, D.PREV_ALU_OUT, D.PREV_DELAY_3) \
        .pass_through_delay(0, 2, 4, 5)
    dp[4].enable_alu(UAluOp.IS_GE, D.PREV_DELAY_0, D.PREV_DELAY_4) \
        .pass_through_delay(2, 4, 5) \
        .enable_delay_from_src(DelayInp.PREV_ALU_OUT, 3)
    dp[5].enable_alu(UAluOp.MULTIPLY, D.PREV_ALU_OUT, D.PREV_DELAY_4) \
        .pass_through_delay(2, 3, 5)
    dp[6].enable_alu(UAluOp.MULTIPLY, D.PREV_ALU_OUT, D.PREV_DELAY_2) \
        .pass_through_delay(3, 5)
    dp[7].enable_alu(UAluOp.ADD, D.PREV_ALU_OUT, D.PREV_DELAY_5) \
        .pass_through_delay(3)
    u2.datapath_config = dp
    u2.out = {OutPath.WR0_LO: OutSel.DELAY_3, OutPath.WR0_HI: OutSel.ALU_OUT,
              OutPath.WR1_LO: OutSel.ALU_OUT, OutPath.WR1_HI: OutSel.ALU_OUT}
    u2.out_enable = {OutPath.WR0_LO: 1, OutPath.WR0_HI: 1,
                     OutPath.WR1_LO: 0, OutPath.WR1_HI: 0}
    return u2


def _register_lif_op():
    name = "LIF_STEP2_ANT"
    for op in dve_ops.OPS:
        if op.name == name:
            return op
    spec = Spec(
        body=(Src0 <= C1) * Src0 * C0 + Src1,
        reference=_lif_ref,
    )
    op = DveOp(name, spec, subdim=False, uops_sha={"v3": "", "v4": ""})
    dve_ops.OPS.append(op)
    row = dve_ops._CUSTOM_DVE_ROW_BASE + len(dve_ops.OPS) - 1
    dve_ops._SUB_OPCODE_FOR_NAME[name] = row
    dve_ops.CUSTOM_DVE_SPECS[name] = spec
    # Inject the compiled spec (with the hand-packed 2X program) into the
    # compile cache so DveOp.compile() and the table generator use it.
    reg = lower(spec, ver="v3")
    compiled = DveOpSpec(name=name, opcode=row, uops=reg,
                         uops_2x=[_build_2x_uop(reg[0])],
                         perf_max=1, rd1_en=True)
    compiled.validate("v3")
    dve_ops._COMPILE_CACHE[(name, "v3")] = compiled
    return op


def _build_nc():
    lif_op = _register_lif_op()
    nc = bacc.Bacc("TRN2", debug=False, num_devices=N_CORES)
    x = nc.dram_tensor("x", [T, S], mybir.dt.float16, kind="ExternalInput").ap()
    y = nc.dram_tensor("y", [T, S], mybir.dt.float8e4, kind="ExternalOutput").ap()
    # pair-of-steps view: xr2[tt] holds steps 2tt and 2tt+1 as [P, 2, F]
    xr2 = x.rearrange("(tt two) (p f) -> tt p two f", two=2, p=P)
    yr = y.rearrange("t (p f) -> t p f", p=P)

    f32 = mybir.dt.float32
    f16 = mybir.dt.float16
    Act = mybir.ActivationFunctionType
    Alu = mybir.AluOpType

    def lif_step(out, in0, in1, s0):
        bi = nc.vector._custom_dve(
            lif_op, out=out, in0=in0, in1=in1, s0=s0, s1=THR,
        )
        bi.ins.perf_max = 1  # byte-36[7:6]: allow the 2X_1PORT table slot
        return bi

    H = F // 2
    with TileContext(nc) as tc:
        with (
            tc.tile_pool(name="vstate", bufs=5) as v_pool,
            tc.tile_pool(name="xin", bufs=6) as xin_pool,
            tc.tile_pool(name="sout", bufs=10) as s_pool,
        ):
            negthr = nc.alloc_sbuf_tensor("const_negthr", [P, 1], f32).ap()
            nc.vector.memset(negthr, -THR)

            v_prev = None
            xt2 = None
            for t in range(T):
                if t % 2 == 0:
                    xt2 = xin_pool.tile([P, 2, F], f16)
                    if t == 0:
                        # soft-start: x_0 halves race down both HWDGE rings,
                        # then x_1..x_3 as single-step loads so early compute
                        # is never gated on a full 1 MiB pair transfer
                        nc.sync.dma_start(out=xt2[:, 0, :H], in_=xr2[0][:, 0, :H])
                        nc.scalar.dma_start(out=xt2[:, 0, H:], in_=xr2[0][:, 0, H:])
                        nc.sync.dma_start(out=xt2[:, 1, :H], in_=xr2[0][:, 1, :H])
                        nc.sync.dma_start(out=xt2[:, 1, H:], in_=xr2[0][:, 1, H:])
                    elif t == 2:
                        nc.sync.dma_start(out=xt2[:, 0], in_=xr2[1][:, 0])
                        nc.sync.dma_start(out=xt2[:, 1], in_=xr2[1][:, 1])
                    else:
                        # quarter-granular loads: spread SBUF write bursts so
                        # they contend less with the DVE's reads
                        xq = xr2[t // 2]
                        nc.sync.dma_start(out=xt2[:, 0, :H], in_=xq[:, 0, :H])
                        nc.sync.dma_start(out=xt2[:, 0, H:], in_=xq[:, 0, H:])
                        nc.sync.dma_start(out=xt2[:, 1, :H], in_=xq[:, 1, :H])
                        nc.sync.dma_start(out=xt2[:, 1, H:], in_=xq[:, 1, H:])
                xt = xt2[:, t % 2]

                v = v_pool.tile([P, F], f16)
                st = s_pool.tile([P, F], mybir.dt.float8e4)
                if t == 0:
                    # v_0 = x_0: same op with decay scalar 0 (mask*v*0 + x = x);
                    # halves so compute starts as soon as half of x_0 landed
                    for c0, c1 in ((0, H), (H, F)):
                        lif_step(v[:, c0:c1], xt[:, c0:c1], xt[:, c0:c1], 0.0)
                elif t == 1 or t == T - 1:
                    # head/tail latency trim: process in column halves
                    for c0, c1 in ((0, H), (H, F)):
                        lif_step(v[:, c0:c1], v_prev[:, c0:c1], xt[:, c0:c1], DECAY)
                else:
                    lif_step(v[:], v_prev[:], xt, DECAY)
                # spike encoding; host decodes (y > 0) either way:
                #  - steady state: DVE (v > THR) -> {0,1} fp8 for cols [0,SW),
                #    ScalarE Sign(v-THR) -> {-1,0,1} fp8 for the rest; out-DMA
                #    on ScalarE's HWDGE ring (never blocks the SP input stream)
                #  - t=T-1: all on the DVE, out via SP, so the tail never
                #    waits on the Act engine
                if t == T - 1:
                    for c0, c1 in ((0, H), (H, F)):
                        nc.vector.tensor_scalar(
                            out=st[:, c0:c1], in0=v[:, c0:c1],
                            scalar1=THR, scalar2=None, op0=Alu.is_gt,
                        )
                        nc.sync.dma_start(out=yr[t][:, c0:c1], in_=st[:, c0:c1])
                else:
                    nc.vector.tensor_scalar(
                        out=st[:, :SW], in0=v[:, :SW],
                        scalar1=THR, scalar2=None, op0=Alu.is_gt,
                    )
                    nc.scalar.activation(
                        st[:, SW:], v[:, SW:], Act.Sign, bias=negthr
                    )
                    nc.scalar.dma_start(out=yr[t], in_=st[:])
                v_prev = v
    nc.compile()
    return nc


LAST_RESULTS = None


def kernel(tx):
    global LAST_RESULTS
    tx = np.asarray(tx)
    assert tx.shape == (T, B, N) and tx.dtype == np.float32

    if "nc" not in _BUILT:
        _BUILT["nc"] = _build_nc()
    nc = _BUILT["nc"]

    tx16 = tx.astype(np.float16)
    in_maps = [
        {"x": np.ascontiguousarray(tx16[:, c * B_SH:(c + 1) * B_SH, :]).reshape(T, S)}
        for c in range(N_CORES)
    ]
    res = run_bass_kernel_spmd(nc, in_maps, core_ids=list(range(N_CORES)), trace=TRACE)
    LAST_RESULTS = res

    out = np.empty((T, B, N), dtype=np.float32)
    for c in range(N_CORES):
        sgn = np.asarray(res.results[c]["y"]).reshape(T, B_SH, N)
        out[:, c * B_SH:(c + 1) * B_SH, :] = (sgn > 0).astype(np.float32)
    return out


# revision 33
# speedup vs baseline: 1.0430x; 1.0430x over previous
"""LIF (leaky integrate-and-fire) forward kernel for Trainium2, 8 NeuronCores.

Recurrence (per element of [B, N], serial over T):
    v_t = DECAY * v_{t-1} * (1 - s_{t-1}) + x_t
    s_t = (v_t > THRESHOLD)

State carried as v (pre-reset membrane, fp16). One fused custom-DVE op per
step (registered at build time via the public dve_ops extension point):
    v_t = (v_{t-1} <= THR) * v_{t-1} * DECAY + x_t
with a hand-packed 2X_1PORT uop variant (element A on ALU slices 0-3,
element B from the SRC_*_HI packed fp16 halves on slices 4-7) and
perf_max=1 set on each emitted instruction so the engine may engage it.
Spikes: DVE (v > THR) -> {0,1} fp8 for the first SW columns, ScalarE
Sign(v-THR) -> {-1,0,1} fp8 for the rest; host decodes (y > 0) for both.
Inputs stream as step-pairs on the SP HWDGE ring; output DMA is issued from
the Activation engine's HWDGE ring so it never head-of-line blocks the
input stream (final step drains via the then-idle SP ring).

Input is converted to fp16 on the host (halves input HBM traffic; measured
spike-flip impact 2211/67M elements, rel err 0.0097 < 2e-2 gate).

Sharding: batch dim (128) split 16 rows/core across 8 cores; per-core,
per-step slab is a contiguous block viewed as [128 partitions, 2048].
"""

import copy

import numpy as np

import concourse.bacc as bacc
import concourse.mybir as mybir
from concourse.tile import TileContext
from concourse.bass_utils import run_bass_kernel_spmd
import concourse.dve_ops as dve_ops
from concourse.dve_ops import DveOp
from concourse.dve_spec import Spec, Src0, Src1, C0, C1, lower
from concourse.dve_uop import (
    AluInp, AluOp as UAluOp, DelayInp, DveOpSpec, InpSel, OutPath, OutSel,
    UopDpConfig,
)

T, B, N = 32, 128, 16384
N_CORES = 8
B_SH = B // N_CORES          # 16 batch rows per core
S = B_SH * N                 # 262144 elements per core per time step
P = 128                      # SBUF partitions
F = S // P                   # 2048 free-dim elements
SW = 768                     # spike columns computed on DVE (rest on Act)
DECAY = 0.2
THR = 0.3

TRACE = False                # set True (e.g. from test.py) to capture a profile

_BUILT = {}


def _lif_ref(in0, in1, s0, s1, imm2):
    v = np.where(np.asarray(in0, dtype=np.float32) <= np.float32(s1),
                 np.asarray(in0, dtype=np.float32), np.float32(0.0))
    return v * np.float32(s0) + np.asarray(in1, dtype=np.float32)


def _build_2x_uop(reg_uop):
    """Hand-packed 2X_1PORT variant: element A on ALU slices 0-3, element B
    (from the SRC_*_HI packed half) on slices 4-7; A's result rides delay
    lane 3 from stage 4 and is emitted on WR0_LO, B's on WR0_HI."""
    u2 = copy.deepcopy(reg_uop)
    u2.inp = [InpSel.ZERO] * 8
    u2.inp_enable = [0] * 8
    for sel, slot in ((InpSel.CONST_1, 1), (InpSel.SRC_0, 2), (InpSel.CONST_0, 3),
                      (InpSel.SRC_1, 4), (InpSel.SRC_0_HI, 5), (InpSel.SRC_1_HI, 6)):
        u2.enable_input(sel, slot)
    D = AluInp
    dp = [UopDpConfig() for _ in range(8)]
    dp[0].enable_alu(UAluOp.IS_GE, D.PREV_DELAY_0, D.PREV_DELAY_1) \
        .pass_through_delay(0, 1, 2, 3, 4, 5)
    dp[1].enable_alu(UAluOp.MULTIPLY, D.PREV_ALU_OUT, D.PREV_DELAY_1) \
        .pass_through_delay(0, 2, 3, 4, 5)
    dp[2].enable_alu(UAluOp.MULTIPLY, D.PREV_ALU_OUT, D.PREV_DELAY_2) \
        .pass_through_delay(0, 2, 3, 4, 5)
    dp[3].enable_alu(UAluOp.ADD, D.PREV_ALU_OUT, D.PREV_DELAY_3) \
        .pass_through_delay(0, 2, 4, 5)
    dp[4].enable_alu(UAluOp.IS_GE, D.PREV_DELAY_0, D.PREV_DELAY_4) \
        .pass_through_delay(2, 4, 5) \
        .enable_delay_from_src(DelayInp.PREV_ALU_OUT, 3)
    dp[5].enable_alu(UAluOp.MULTIPLY, D.PREV_ALU_OUT, D.PREV_DELAY_4) \
        .pass_through_delay(2, 3, 5)
    dp[6].enable_alu(UAluOp.MULTIPLY, D.PREV_ALU_OUT, D.PREV_DELAY_2) \
        .pass_through_delay(3, 5)
    dp[7].enable_alu(UAluOp.ADD, D.PREV_ALU_OUT, D.PREV_DELAY_5) \
        .pass_through_delay(3)
    u2.datapath_config = dp
    u2.out = {OutPath.WR0_LO: OutSel.DELAY_3, OutPath.WR0_HI: OutSel.ALU_OUT,
              OutPath.WR1_LO: OutSel.ALU_OUT, OutPath.WR1_HI: OutSel.ALU_OUT}
    u2.out_enable = {OutPath.WR0_LO: 1, OutPath.WR0_HI: 1,
                     OutPath.WR1_LO: 0, OutPath.WR1_HI: 0}
    return u2


def _register_lif_op():
    name = "LIF_STEP2_ANT"
    for op in dve_ops.OPS:
        if op.name == name:
            return op
    spec = Spec(
        body=(Src0 <= C1) * Src0 * C0 + Src1,
        reference=_lif_ref,
    )
    op = DveOp(name, spec, subdim=False, uops_sha={"v3": "", "v4": ""})
    dve_ops.OPS.append(op)
    row = dve_ops._CUSTOM_DVE_ROW_BASE + len(dve_ops.OPS) - 1
    dve_ops._SUB_OPCODE_FOR_NAME[name] = row
    dve_ops.CUSTOM_DVE_SPECS[name] = spec
    # Inject the compiled spec (with the hand-packed 2X program) into the
    # compile cache so DveOp.compile() and the table generator use it.
    reg = lower(spec, ver="v3")
    compiled = DveOpSpec(name=name, opcode=row, uops=reg,
                         uops_2x=[_build_2x_uop(reg[0])],
                         perf_max=1, rd1_en=True)
    compiled.validate("v3")
    dve_ops._COMPILE_CACHE[(name, "v3")] = compiled
    return op


def _build_nc():
    lif_op = _register_lif_op()
    nc = bacc.Bacc("TRN2", debug=False, num_devices=N_CORES)
    x = nc.dram_tensor("x", [T, S], mybir.dt.float16, kind="ExternalInput").ap()
    y = nc.dram_tensor("y", [T, S], mybir.dt.float8e4, kind="ExternalOutput").ap()
    # pair-of-steps view: xr2[tt] holds steps 2tt and 2tt+1 as [P, 2, F]
    xr2 = x.rearrange("(tt two) (p f) -> tt p two f", two=2, p=P)
    yr = y.rearrange("t (p f) -> t p f", p=P)

    f32 = mybir.dt.float32
    f16 = mybir.dt.float16
    Act = mybir.ActivationFunctionType
    Alu = mybir.AluOpType

    def lif_step(out, in0, in1, s0):
        bi = nc.vector._custom_dve(
            lif_op, out=out, in0=in0, in1=in1, s0=s0, s1=THR,
        )
        bi.ins.perf_max = 1  # byte-36[7:6]: allow the 2X_1PORT table slot
        return bi

    H = F // 2
    with TileContext(nc) as tc:
        with (
            tc.tile_pool(name="vstate", bufs=5) as v_pool,
            tc.tile_pool(name="xin", bufs=6) as xin_pool,
            tc.tile_pool(name="sout", bufs=10) as s_pool,
        ):
            negthr = nc.alloc_sbuf_tensor("const_negthr", [P, 1], f32).ap()
            nc.vector.memset(negthr, -THR)

            v_prev = None
            xt2 = None
            for t in range(T):
                if t % 2 == 0:
                    xt2 = xin_pool.tile([P, 2, F], f16)
                    if t == 0:
                        # soft-start: x_0 halves race down both HWDGE rings,
                        # then x_1..x_3 as single-step loads so early compute
                        # is never gated on a full 1 MiB pair transfer
                        nc.sync.dma_start(out=xt2[:, 0, :H], in_=xr2[0][:, 0, :H])
                        nc.scalar.dma_start(out=xt2[:, 0, H:], in_=xr2[0][:, 0, H:])
                        nc.sync.dma_start(out=xt2[:, 1, :H], in_=xr2[0][:, 1, :H])
                        nc.sync.dma_start(out=xt2[:, 1, H:], in_=xr2[0][:, 1, H:])
                    elif t == 2:
                        nc.sync.dma_start(out=xt2[:, 0], in_=xr2[1][:, 0])
                        nc.sync.dma_start(out=xt2[:, 1], in_=xr2[1][:, 1])
                    else:
                        nc.sync.dma_start(out=xt2[:], in_=xr2[t // 2])
                xt = xt2[:, t % 2]

                v = v_pool.tile([P, F], f16)
                st = s_pool.tile([P, F], mybir.dt.float8e4)
                if t == 0:
                    # v_0 = x_0: same op with decay scalar 0 (mask*v*0 + x = x);
                    # halves so compute starts as soon as half of x_0 landed
                    for c0, c1 in ((0, H), (H, F)):
                        lif_step(v[:, c0:c1], xt[:, c0:c1], xt[:, c0:c1], 0.0)
                elif t == 1 or t == T - 1:
                    # head/tail latency trim: process in column halves
                    for c0, c1 in ((0, H), (H, F)):
                        lif_step(v[:, c0:c1], v_prev[:, c0:c1], xt[:, c0:c1], DECAY)
                else:
                    lif_step(v[:], v_prev[:], xt, DECAY)
                # spike encoding; host decodes (y > 0) either way:
                #  - steady state: DVE (v > THR) -> {0,1} fp8 for cols [0,SW),
                #    ScalarE Sign(v-THR) -> {-1,0,1} fp8 for the rest; out-DMA
                #    on ScalarE's HWDGE ring (never blocks the SP input stream)
                #  - t=T-1: all on the DVE, out via SP, so the tail never
                #    waits on the Act engine
                if t == T - 1:
                    for c0, c1 in ((0, H), (H, F)):
                        nc.vector.tensor_scalar(
                            out=st[:, c0:c1], in0=v[:, c0:c1],
                            scalar1=THR, scalar2=None, op0=Alu.is_gt,
                        )
                        nc.sync.dma_start(out=yr[t][:, c0:c1], in_=st[:, c0:c1])
                else:
                    nc.vector.tensor_scalar(
                        out=st[:, :SW], in0=v[:, :SW],
                        scalar1=THR, scalar2=None, op0=Alu.is_gt,
                    )
                    nc.scalar.activation(
                        st[:, SW:], v[:, SW:], Act.Sign, bias=negthr
                    )
                    nc.scalar.dma_start(out=yr[t], in_=st[:])
                v_prev = v
    nc.compile()
    return nc


LAST_RESULTS = None


def kernel(tx):
    global LAST_RESULTS
    tx = np.asarray(tx)
    assert tx.shape == (T, B, N) and tx.dtype == np.float32

    if "nc" not in _BUILT:
        _BUILT["nc"] = _build_nc()
    nc = _BUILT["nc"]

    tx16 = tx.astype(np.float16)
    in_maps = [
        {"x": np.ascontiguousarray(tx16[:, c * B_SH:(c + 1) * B_SH, :]).reshape(T, S)}
        for c in range(N_CORES)
    ]
    res = run_bass_kernel_spmd(nc, in_maps, core_ids=list(range(N_CORES)), trace=TRACE)
    LAST_RESULTS = res

    out = np.empty((T, B, N), dtype=np.float32)
    for c in range(N_CORES):
        sgn = np.asarray(res.results[c]["y"]).reshape(T, B_SH, N)
        out[:, c * B_SH:(c + 1) * B_SH, :] = (sgn > 0).astype(np.float32)
    return out
